# revision 47
# baseline (speedup 1.0000x reference)
"""Trainium2 Bass kernel for nn_AttnBlock (GroupNorm + single-head 4096-token
attention + residual), sharded over 8 NeuronCores.

Sharding: data-parallel over batch B=4, sequence-parallel x2 over the 4096
query tokens -> 8 shards. Each core computes k/v for its full batch
(duplicated across the 2 token-halves) and q/attention/out-proj for its 2048
query tokens. The token axis is rolled on the host for the second half so a
single SPMD NEFF serves all cores (softmax over keys is order-invariant,
groupnorm stats are token-permutation-invariant).

v3 pipeline: all large matmuls are fp8(e4m3) MatmulPerfMode.DoubleRow
(K=256/instr). The ACT engine's exp stream is the spine: pT (exp scores) is
double-buffered across strips so exps never wait on downstream consumers.
Strip st's h/l matmuls run inside strip st+1's score window; the v
projection hides inside strip 0's score window. The softmax denominator l
(M=1 ones-matmul over the quantized pT tiles) normalizes h at evacuation.
The v bias is folded into bo on the host (softmax weights sum to 1).
x stays resident in SBUF for the residual. PSUM->SBUF evacuations use
per-engine psum pools so ACT and DVE drain in parallel.

Self-contained: hardcodes all shapes; only needs the concourse runtime.
"""

import numpy as np
import ml_dtypes

import concourse.bass as bass
import concourse.bacc as bacc
import concourse.tile as tile
from concourse import mybir
from concourse.bass_utils import run_bass_kernel_spmd

P = 128                 # partitions
C = 512                 # channels
N = 4096                # tokens (64*64)
NQ = 2048               # query tokens per core
CT = C // P             # 4 channel tiles of 128
CP = 2                  # channel pair-tiles (DoubleRow K=256)
JT = N // P             # 32 key-token tiles of 128
JP = JT // 2            # 16 key-token pair-tiles
NSTRIP = NQ // 512      # 4 query strips of 512
GS = 16                 # channels per group
NG = P // GS            # 8 groups per channel tile
EPS = 1e-6
SCALE = float(C) ** -0.5
EXP_BIAS = -2.5         # keeps unnormalized h inside fp8-e4m3 range (240)
V_SCALE = 0.125         # v stored as v/8 in fp8; wo scaled x8 on the host
F32 = mybir.dt.float32
BF16 = mybir.dt.bfloat16
F8 = mybir.dt.float8e4
DR = mybir.MatmulPerfMode.DoubleRow
ADD = mybir.AluOpType.add
MULT = mybir.AluOpType.mult
IDENT = mybir.ActivationFunctionType.Identity
EXP = mybir.ActivationFunctionType.Exp

_CACHE = {}


def build_bass(debug=False):
    nc = bacc.Bacc(None, target_bir_lowering=False)

    x_h = nc.dram_tensor("x", [C, N], F32, kind="ExternalInput")[:]
    wq_h = nc.dram_tensor("wqT", [C, C], F8, kind="ExternalInput")[:]
    wk_h = nc.dram_tensor("wkT", [C, C], F8, kind="ExternalInput")[:]
    wv_h = nc.dram_tensor("wvT", [C, C], F8, kind="ExternalInput")[:]
    wo_h = nc.dram_tensor("woT", [C, C], F8, kind="ExternalInput")[:]
    # all per-channel vectors pre-shaped on the host into one [128, 28]
    # tensor (col-major channel blocks): one contiguous DMA instead of five
    # 512-descriptor gathers. cols: bq bk bo gam bet (4 each), g8 (8)
    cvec_h = nc.dram_tensor("cvec", [P, 28], F32, kind="ExternalInput")[:]
    out_h = nc.dram_tensor("out", [C, NQ], F32, kind="ExternalOutput")[:]

    dbg = {}
    if debug:
        dbg["hn"] = nc.dram_tensor("d_hn", [CP, P, 2, N], F8, kind="ExternalOutput")[:]
        dbg["q"] = nc.dram_tensor("d_q", [CP, P, 2, NQ], F8, kind="ExternalOutput")[:]
        dbg["k"] = nc.dram_tensor("d_k", [CP, P, 2, N], F8, kind="ExternalOutput")[:]
        dbg["v"] = nc.dram_tensor("d_v", [JP, P, 2, C], F8, kind="ExternalOutput")[:]
        dbg["hT"] = nc.dram_tensor("d_hT", [CP, P, 2, NQ], F8, kind="ExternalOutput")[:]

    g8T_np = np.zeros((NG, P), np.float32)
    for c in range(P):
        g8T_np[c // GS, c] = 1.0
    g8T_h = nc.inline_tensor(g8T_np, name="g8T")[:]

    x_t = x_h.rearrange("(t p) n -> t p n", p=P)          # [4,128,4096]
    out_t = out_h.rearrange("(t p) n -> t p n", p=P)      # [4,128,2048]

    with tile.TileContext(nc) as tc:
        with tc.tile_pool(name="consts", bufs=1) as cp, \
             tc.tile_pool(name="wgt", bufs=1) as wp, \
             tc.tile_pool(name="xres", bufs=1) as xp, \
             tc.tile_pool(name="qkv", bufs=1) as qkvp, \
             tc.tile_pool(name="hT", bufs=1) as hTp:

            # ---- constants ----
            eps_t = cp.tile([P, 1], F32, tag="eps")
            nc.vector.memset(eps_t[:], EPS)
            ebias_t = cp.tile([P, 1], F32, tag="ebias")
            nc.vector.memset(ebias_t[:], EXP_BIAS)
            # DoubleRow ldweights needs the k-pair dim step to be a multiple
            # of 16 bytes, so pad the ones column out to 16
            ones_f8 = cp.tile([P, 2, 16], F8, tag="ones8")
            nc.vector.memset(ones_f8[:], 1.0)
            cvec_sb = cp.tile([P, 28], F32, tag="cvec")
            g8T_sb = cp.tile([NG, P], F32, tag="g8T")

            # ---- persistent activations (fp8, DoubleRow pair layout) ----
            x_sb = [xp.tile([P, N], F32, tag=f"x{t}", name=f"x{t}")
                    for t in range(CT)]
            hn_f8 = [qkvp.tile([P, 2, N], F8, tag=f"hn{t}", name=f"hn{t}")
                     for t in range(CP)]
            q_f8 = [qkvp.tile([P, 2, NQ], F8, tag=f"q{t}", name=f"q{t}")
                    for t in range(CP)]
            k_f8 = [qkvp.tile([P, 2, N], F8, tag=f"k{t}", name=f"k{t}")
                    for t in range(CP)]
            v_f8 = [qkvp.tile([P, 2, C], F8, tag=f"v{j}", name=f"v{j}")
                    for j in range(JP)]
            hT_f8 = [hTp.tile([P, 2, NQ], F8, tag=f"hT{t}", name=f"hT{t}")
                     for t in range(CP)]
            w_sb = {}
            for wname in ("wq", "wk", "wv", "wo"):
                w_sb[wname] = [wp.tile([P, 2, C], F8, tag=f"{wname}{t}",
                                       name=f"{wname}{t}") for t in range(CP)]

            # =========== Phase A: groupnorm -> hn (fp8) ===========
            with tc.tile_pool(name="gnsb", bufs=2) as gnp, \
                 tc.tile_pool(name="gnps", bufs=2, space="PSUM") as gnps:

                for ct in range(CT):
                    stats = gnp.tile([P, 8, 6], F32, tag="stats")
                    for s in range(8):
                        nc.sync.dma_start(
                            out=x_sb[ct][:, s * 512:(s + 1) * 512],
                            in_=x_t[ct][:, s * 512:(s + 1) * 512],
                        )
                        nc.vector.bn_stats(
                            out=stats[:, s, :], in_=x_sb[ct][:, s * 512:(s + 1) * 512]
                        )
                    if ct == 0:
                        # consts ride the DMA queue behind ct0's x chunks
                        nc.sync.dma_start(out=cvec_sb[:], in_=cvec_h)
                        nc.sync.dma_start(out=g8T_sb[:], in_=g8T_h)
                    mv = gnp.tile([P, 2], F32, tag="mv")
                    nc.vector.bn_aggr(out=mv[:], in_=stats[:])
                    cstat = gnp.tile([P, 2], F32, tag="cstat")
                    nc.vector.tensor_copy(cstat[:, 0:1], mv[:, 0:1])
                    nc.vector.tensor_mul(cstat[:, 1:2], mv[:, 0:1], mv[:, 0:1])
                    nc.vector.tensor_add(cstat[:, 1:2], cstat[:, 1:2], mv[:, 1:2])
                    psA = gnps.tile([NG, 2], F32, tag="gn")
                    nc.tensor.matmul(psA[:], lhsT=cvec_sb[:, 20:28], rhs=cstat[:],
                                     start=True, stop=True)
                    gt = gnp.tile([NG, 2], F32, tag="gt")
                    nc.vector.tensor_copy(gt[:], psA[:])
                    psB = gnps.tile([P, 2], F32, tag="gn")
                    nc.tensor.matmul(psB[:], lhsT=g8T_sb[:], rhs=gt[:],
                                     start=True, stop=True)
                    gstat = gnp.tile([P, 2], F32, tag="gstat")
                    nc.vector.tensor_copy(gstat[:], psB[:])
                    vtmp = gnp.tile([P, 1], F32, tag="vtmp")
                    nc.vector.tensor_mul(vtmp[:], gstat[:, 0:1], gstat[:, 0:1])
                    nc.vector.tensor_tensor(
                        out=vtmp[:], in0=gstat[:, 1:2], in1=vtmp[:],
                        op=mybir.AluOpType.subtract,
                    )
                    nc.scalar.activation(
                        out=vtmp[:], in_=vtmp[:],
                        func=mybir.ActivationFunctionType.Sqrt,
                        bias=eps_t[:], scale=1.0,
                    )
                    rstd = gnp.tile([P, 1], F32, tag="rstd")
                    nc.vector.reciprocal(out=rstd[:], in_=vtmp[:])
                    a_t = gnp.tile([P, 1], F32, tag="a_t")
                    nc.vector.tensor_mul(a_t[:], rstd[:], cvec_sb[:, 12 + ct:13 + ct])
                    d_t = gnp.tile([P, 1], F32, tag="d_t")
                    nc.vector.tensor_mul(d_t[:], gstat[:, 0:1], a_t[:])
                    nc.vector.tensor_tensor(
                        out=d_t[:], in0=cvec_sb[:, 16 + ct:17 + ct], in1=d_t[:],
                        op=mybir.AluOpType.subtract,
                    )
                    # apply split ACT/DVE so the last tile's apply is short
                    hdst = hn_f8[ct // 2]
                    nc.scalar.activation(
                        out=hdst[:, ct % 2, 0:2048],
                        in_=x_sb[ct][:, 0:2048],
                        func=IDENT, scale=a_t[:], bias=d_t[:],
                    )
                    nc.vector.tensor_scalar(
                        out=hdst[:, ct % 2, 2048:4096],
                        in0=x_sb[ct][:, 2048:4096],
                        scalar1=a_t[:], scalar2=d_t[:],
                        op0=MULT, op1=ADD,
                    )

            # deferred weight loads (after x so groupnorm owns DMA at t=0)
            wq_t = wq_h.rearrange("(t p) o -> t p o", p=P)
            wk_t = wk_h.rearrange("(t p) o -> t p o", p=P)
            wv_t = wv_h.rearrange("(t p) o -> t p o", p=P)
            wo_t = wo_h.rearrange("(t p) o -> t p o", p=P)
            for t in range(CP):
                for s in range(2):
                    nc.sync.dma_start(out=w_sb["wq"][t][:, s, :], in_=wq_t[2 * t + s])
                    nc.sync.dma_start(out=w_sb["wk"][t][:, s, :], in_=wk_t[2 * t + s])
                    nc.sync.dma_start(out=w_sb["wv"][t][:, s, :], in_=wv_t[2 * t + s])
                    nc.sync.dma_start(out=w_sb["wo"][t][:, s, :], in_=wo_t[2 * t + s])

            # =========== Phase B: k/q projections (fp8 DoubleRow) ===========
            # Per-engine psum pools (ACT and DVE drain their own rings in
            # parallel); 2-bank tiles pairing adjacent token slices of the
            # same out-channel block so the evac is one wide instruction.
            with tc.tile_pool(name="pjA", bufs=2, space="PSUM") as pjA, \
                 tc.tile_pool(name="pjD", bufs=2, space="PSUM") as pjD:

                def proj_pair(idx, wname, osl2, co, dst, bcol):
                    on_act = idx % 2 == 0
                    pool = pjA if on_act else pjD
                    ps = pool.tile([P, 1024], F32, tag="pj")
                    for h_ in range(2):
                        for t in range(CP):
                            nc.tensor.matmul(
                                ps[:, h_ * 512:(h_ + 1) * 512],
                                lhsT=w_sb[wname][t][:, :, co * P:(co + 1) * P],
                                rhs=hn_f8[t][:, :, (osl2 * 2 + h_) * 512:
                                             (osl2 * 2 + h_ + 1) * 512],
                                start=(t == 0), stop=(t == CP - 1),
                                perf_mode=DR,
                            )
                    if on_act:
                        nc.scalar.activation(out=dst, in_=ps[:], func=IDENT,
                                             bias=bcol, scale=1.0)
                    else:
                        nc.vector.tensor_scalar_add(out=dst, in0=ps[:],
                                                    scalar1=bcol)

                ei = 0
                # k for all 4096 keys, (jsl2, co) order so early score tiles
                # unblock first; then q
                for jsl2 in range(N // 1024):
                    for co in range(CT):
                        proj_pair(ei, "wk", jsl2, co,
                                  k_f8[co // 2][:, co % 2, jsl2 * 1024:(jsl2 + 1) * 1024],
                                  cvec_sb[:, 4 + co:5 + co])
                        ei += 1
                for isl2 in range(NQ // 1024):
                    for co in range(CT):
                        proj_pair(ei, "wq", isl2, co,
                                  q_f8[co // 2][:, co % 2, isl2 * 1024:(isl2 + 1) * 1024],
                                  cvec_sb[:, 0 + co:1 + co])
                        ei += 1

            # =========== Phase C: attention pipeline ===========
            # pT is double-buffered across strips so the ACT exp stream never
            # waits for consumers. Strip st's l and h matmuls run inside
            # strip st+1's score window; h accumulates CB-MAJOR (one output
            # channel block at a time over all 16 resident pT pairs), which
            # needs only a 2-bank ping-pong instead of 4 held banks. The v
            # projection hides inside strip 0's window; its psum pool closes
            # before the h pools open so everything fits in 8 banks.
            with tc.tile_pool(name="scA", bufs=2, space="PSUM") as scA, \
                 tc.tile_pool(name="attn", bufs=1) as ap_, \
                 tc.tile_pool(name="lsb", bufs=2) as lsp, \
                 tc.tile_pool(name="outt", bufs=3) as otp:

                # two pT sets (strip parity)
                pT = [[ap_.tile([P, 2, 512], F8, tag=f"pT{s}_{j}",
                                name=f"pT{s}_{j}") for j in range(JP)]
                      for s in range(2)]

                def sc_slot(st, jp):
                    """One 2-bank score pair tile + its exp."""
                    i0 = st * 512
                    sc = scA.tile([P, 1024], F32, tag="scA",
                                  name=f"s{st}_{jp}")
                    for h_ in range(2):
                        for t in range(CP):
                            nc.tensor.matmul(
                                sc[:, h_ * 512:(h_ + 1) * 512],
                                lhsT=k_f8[t][:, :, (2 * jp + h_) * P:(2 * jp + h_ + 1) * P],
                                rhs=q_f8[t][:, :, i0:i0 + 512],
                                start=(t == 0), stop=(t == CP - 1),
                                perf_mode=DR,
                            )
                    nc.scalar.activation(
                        out=pT[st % 2][jp][:], in_=sc[:],
                        func=EXP, scale=SCALE, bias=ebias_t[:],
                    )

                def aux_v(pjV):
                    """v projection: matmuls on PE, scaled-copy evac on DVE
                    (bv folded into bo on the host)."""
                    for jp in range(JP):
                        ps = pjV.tile([P, 1024], F32, tag="pv", name=f"v{jp}")
                        for m in range(2):
                            for t in range(CP):
                                yield nc.tensor.matmul(
                                    ps[:, m * 512:(m + 1) * 512],
                                    lhsT=hn_f8[t][:, :, (2 * jp + m) * P:(2 * jp + m + 1) * P],
                                    rhs=w_sb["wv"][t][:],
                                    start=(t == 0), stop=(t == CP - 1),
                                    perf_mode=DR,
                                )
                        nc.vector.tensor_scalar_mul(out=v_f8[jp][:], in0=ps[:],
                                                    scalar1=V_SCALE)

                def aux_lh(st, hp, lpool):
                    """Deferred work for strip st (runs in strip st+1's
                    window): l-run, rl, rlb, then cb-major h runs with
                    normalized fp8 evacs."""
                    i0 = st * 512
                    pts = pT[st % 2]
                    lt = lpool.tile([1, 512], F32, tag="l", name=f"l{st}")
                    for jp in range(JP):
                        yield nc.tensor.matmul(
                            lt[:], lhsT=ones_f8[:, :, 0:1], rhs=pts[jp][:],
                            start=(jp == 0), stop=(jp == JP - 1),
                            perf_mode=DR,
                        )
                    rl1 = lsp.tile([1, 512], F32, tag="rl1", name=f"rl1{st}")
                    nc.vector.reciprocal(out=rl1[:], in_=lt[:])
                    rlb = lsp.tile([P, 512], F32, tag="rlb", name=f"rlb{st}")
                    nc.gpsimd.partition_broadcast(rlb[:], rl1[:])
                    for cb in range(CT):
                        hps = hp.tile([P, 512], F32, tag="h",
                                      name=f"hps{st}_{cb}")
                        for jp in range(JP):
                            yield nc.tensor.matmul(
                                hps[:],
                                lhsT=v_f8[jp][:, :, cb * P:(cb + 1) * P],
                                rhs=pts[jp][:],
                                start=(jp == 0), stop=(jp == JP - 1),
                                perf_mode=DR,
                            )
                        nc.vector.tensor_mul(
                            hT_f8[cb // 2][:, cb % 2, i0:i0 + 512],
                            hps[:], rlb[:],
                        )

                def strip_out(st, hp):
                    """out-projection + bias + residual + store."""
                    i0 = st * 512
                    for co in range(CT):
                        ps = hp.tile([P, 512], F32, tag="h", name=f"op{st}_{co}")
                        for t in range(CP):
                            nc.tensor.matmul(
                                ps[:],
                                lhsT=w_sb["wo"][t][:, :, co * P:(co + 1) * P],
                                rhs=hT_f8[t][:, :, i0:i0 + 512],
                                start=(t == 0), stop=(t == CP - 1),
                                perf_mode=DR,
                            )
                        ot = otp.tile([P, 512], F32, tag="ot")
                        nc.vector.scalar_tensor_tensor(
                            out=ot[:], in0=ps[:], scalar=cvec_sb[:, 8 + co:9 + co],
                            in1=x_sb[co][:, i0:i0 + 512], op0=ADD, op1=ADD,
                        )
                        nc.sync.dma_start(
                            out=out_t[co][:, i0:i0 + 512], in_=ot[:]
                        )

                def weave(st, aux_gen):
                    """Emit strip st's 16 score slots with ~5 aux PE ops
                    between consecutive slots."""
                    for jp in range(JP):
                        sc_slot(st, jp)
                        if aux_gen is not None:
                            for _ in range(6 if st == 0 else 5):
                                if next(aux_gen, None) is None:
                                    aux_gen = None
                                    break
                    while aux_gen is not None and next(aux_gen, None) is not None:
                        pass

                # strip 0 (v hides in its window; pjV closes right after)
                pjV_cm = tc.tile_pool(name="pjV", bufs=2, space="PSUM")
                pjV = pjV_cm.__enter__()
                weave(0, aux_v(pjV))
                pjV_cm.__exit__(None, None, None)

                hp_cm = tc.tile_pool(name="hacc", bufs=2, space="PSUM")
                hp = hp_cm.__enter__()
                lp_cm = tc.tile_pool(name="lps", bufs=1, space="PSUM")
                lpool = lp_cm.__enter__()

                for st in range(1, NSTRIP):
                    weave(st, aux_lh(st - 1, hp, lpool))
                    strip_out(st - 1, hp)
                # drain: last strip's deferred work
                for _ in aux_lh(NSTRIP - 1, hp, lpool):
                    pass
                strip_out(NSTRIP - 1, hp)

                lp_cm.__exit__(None, None, None)
                hp_cm.__exit__(None, None, None)

            if debug:
                for t in range(CP):
                    nc.sync.dma_start(out=dbg["hn"][t], in_=hn_f8[t][:])
                    nc.sync.dma_start(out=dbg["q"][t], in_=q_f8[t][:])
                    nc.sync.dma_start(out=dbg["k"][t], in_=k_f8[t][:])
                    nc.sync.dma_start(out=dbg["hT"][t], in_=hT_f8[t][:])
                for jp in range(JP):
                    nc.sync.dma_start(out=dbg["v"][jp], in_=v_f8[jp][:])

    nc.finalize()
    return nc


def kernel(**inputs):
    if "nc" not in _CACHE:
        _CACHE["nc"] = build_bass()
    nc = _CACHE["nc"]

    x = np.ascontiguousarray(np.asarray(inputs["x"], dtype=np.float32))
    B = x.shape[0]
    xf = x.reshape(B, C, N)

    def f8T(w, scale=1.0):
        return np.ascontiguousarray(
            (np.asarray(w, dtype=np.float32).T * scale).astype(
                ml_dtypes.float8_e4m3)
        )

    # softmax weights sum to 1, so the v bias rides through attention:
    # h = p@(v0+bv)/l = p@v0/l + bv  =>  fold wo@bv into bo (exact, fp32)
    wo32 = np.asarray(inputs["wo"], np.float32)
    bo_eff = (np.asarray(inputs["bo"], np.float32)
              + wo32 @ np.asarray(inputs["bv"], np.float32))

    def colsT(v):
        return np.asarray(v, np.float32).reshape(CT, P).T

    g8_np = np.zeros((P, 8), np.float32)
    for c in range(P):
        g8_np[c, c // 16] = 1.0 / 16
    cvec = np.concatenate([
        colsT(inputs["bq"]), colsT(inputs["bk"]), colsT(bo_eff),
        colsT(inputs["norm_g"]), colsT(inputs["norm_b"]), g8_np,
    ], axis=1)

    shared = {
        "wqT": f8T(inputs["wq"]), "wkT": f8T(inputs["wk"]),
        "wvT": f8T(inputs["wv"]), "woT": f8T(inputs["wo"], 1.0 / V_SCALE),
        "cvec": np.ascontiguousarray(cvec, dtype=np.float32),
    }

    in_maps = []
    for core in range(2 * B):
        b, half = core // 2, core % 2
        xb = xf[b]
        if half:
            xb = np.concatenate([xb[:, NQ:], xb[:, :NQ]], axis=1)
        in_maps.append({"x": np.ascontiguousarray(xb), **shared})

    import os
    trace = bool(os.environ.get("BASS_KERNEL_TRACE"))
    res = run_bass_kernel_spmd(
        nc, in_maps, core_ids=list(range(2 * B)), trace=trace,
        trace_cores=list(range(2 * B)) if trace else None,
    )
    _CACHE["last_results"] = res

    out = np.empty((B, C, N), np.float32)
    for core in range(2 * B):
        b, half = core // 2, core % 2
        out[b][:, half * NQ:(half + 1) * NQ] = res.results[core]["out"]
    return out.reshape(B, C, 64, 64)


# revision 49
# speedup vs baseline: 1.0005x; 1.0005x over previous
"""Trainium2 Bass kernel for nn_AttnBlock (GroupNorm + single-head 4096-token
attention + residual), sharded over 8 NeuronCores.

Sharding: data-parallel over batch B=4, sequence-parallel x2 over the 4096
query tokens -> 8 shards. Each core computes k/v for its full batch
(duplicated across the 2 token-halves) and q/attention/out-proj for its 2048
query tokens. The token axis is rolled on the host for the second half so a
single SPMD NEFF serves all cores (softmax over keys is order-invariant,
groupnorm stats are token-permutation-invariant).

v3 pipeline: all large matmuls are fp8(e4m3) MatmulPerfMode.DoubleRow
(K=256/instr). The ACT engine's exp stream is the spine: pT (exp scores) is
double-buffered across strips so exps never wait on downstream consumers.
Strip st's h/l matmuls run inside strip st+1's score window; the v
projection hides inside strip 0's score window. The softmax denominator l
(M=1 ones-matmul over the quantized pT tiles) normalizes h at evacuation.
The v bias is folded into bo on the host (softmax weights sum to 1).
x stays resident in SBUF for the residual. PSUM->SBUF evacuations use
per-engine psum pools so ACT and DVE drain in parallel.

Self-contained: hardcodes all shapes; only needs the concourse runtime.
"""

import numpy as np
import ml_dtypes

import concourse.bass as bass
import concourse.bacc as bacc
import concourse.tile as tile
from concourse import mybir
from concourse.bass_utils import run_bass_kernel_spmd

P = 128                 # partitions
C = 512                 # channels
N = 4096                # tokens (64*64)
NQ = 2048               # query tokens per core
CT = C // P             # 4 channel tiles of 128
CP = 2                  # channel pair-tiles (DoubleRow K=256)
JT = N // P             # 32 key-token tiles of 128
JP = JT // 2            # 16 key-token pair-tiles
NSTRIP = NQ // 512      # 4 query strips of 512
GS = 16                 # channels per group
NG = P // GS            # 8 groups per channel tile
EPS = 1e-6
SCALE = float(C) ** -0.5
EXP_BIAS = -2.5         # keeps unnormalized h inside fp8-e4m3 range (240)
V_SCALE = 0.125         # v stored as v/8 in fp8; wo scaled x8 on the host
F32 = mybir.dt.float32
BF16 = mybir.dt.bfloat16
F8 = mybir.dt.float8e4
DR = mybir.MatmulPerfMode.DoubleRow
ADD = mybir.AluOpType.add
MULT = mybir.AluOpType.mult
IDENT = mybir.ActivationFunctionType.Identity
EXP = mybir.ActivationFunctionType.Exp

_CACHE = {}


def build_bass(debug=False):
    nc = bacc.Bacc(None, target_bir_lowering=False)

    x_h = nc.dram_tensor("x", [C, N], F32, kind="ExternalInput")[:]
    wq_h = nc.dram_tensor("wqT", [C, C], F8, kind="ExternalInput")[:]
    wk_h = nc.dram_tensor("wkT", [C, C], F8, kind="ExternalInput")[:]
    wv_h = nc.dram_tensor("wvT", [C, C], F8, kind="ExternalInput")[:]
    wo_h = nc.dram_tensor("woT", [C, C], F8, kind="ExternalInput")[:]
    # all per-channel vectors pre-shaped on the host into one [128, 28]
    # tensor (col-major channel blocks): one contiguous DMA instead of five
    # 512-descriptor gathers. cols: bq bk bo gam bet (4 each), g8 (8)
    cvec_h = nc.dram_tensor("cvec", [P, 28], F32, kind="ExternalInput")[:]
    out_h = nc.dram_tensor("out", [C, NQ], F32, kind="ExternalOutput")[:]

    dbg = {}
    if debug:
        dbg["hn"] = nc.dram_tensor("d_hn", [CP, P, 2, N], F8, kind="ExternalOutput")[:]
        dbg["q"] = nc.dram_tensor("d_q", [CP, P, 2, NQ], F8, kind="ExternalOutput")[:]
        dbg["k"] = nc.dram_tensor("d_k", [CP, P, 2, N], F8, kind="ExternalOutput")[:]
        dbg["v"] = nc.dram_tensor("d_v", [JP, P, 2, C], F8, kind="ExternalOutput")[:]
        dbg["hT"] = nc.dram_tensor("d_hT", [CP, P, 2, NQ], F8, kind="ExternalOutput")[:]

    g8T_np = np.zeros((NG, P), np.float32)
    for c in range(P):
        g8T_np[c // GS, c] = 1.0
    g8T_h = nc.inline_tensor(g8T_np, name="g8T")[:]

    x_t = x_h.rearrange("(t p) n -> t p n", p=P)          # [4,128,4096]
    out_t = out_h.rearrange("(t p) n -> t p n", p=P)      # [4,128,2048]

    with tile.TileContext(nc) as tc:
        with tc.tile_pool(name="consts", bufs=1) as cp, \
             tc.tile_pool(name="wgt", bufs=1) as wp, \
             tc.tile_pool(name="xres", bufs=1) as xp, \
             tc.tile_pool(name="qkv", bufs=1) as qkvp, \
             tc.tile_pool(name="hT", bufs=1) as hTp:

            # ---- constants ----
            eps_t = cp.tile([P, 1], F32, tag="eps")
            nc.vector.memset(eps_t[:], EPS)
            ebias_t = cp.tile([P, 1], F32, tag="ebias")
            nc.vector.memset(ebias_t[:], EXP_BIAS)
            # DoubleRow ldweights needs the k-pair dim step to be a multiple
            # of 16 bytes, so pad the ones column out to 16
            ones_f8 = cp.tile([P, 2, 16], F8, tag="ones8")
            nc.vector.memset(ones_f8[:], 1.0)
            cvec_sb = cp.tile([P, 28], F32, tag="cvec")
            g8T_sb = cp.tile([NG, P], F32, tag="g8T")

            # ---- persistent activations (fp8, DoubleRow pair layout) ----
            x_sb = [xp.tile([P, N], F32, tag=f"x{t}", name=f"x{t}")
                    for t in range(CT)]
            hn_f8 = [qkvp.tile([P, 2, N], F8, tag=f"hn{t}", name=f"hn{t}")
                     for t in range(CP)]
            q_f8 = [qkvp.tile([P, 2, NQ], F8, tag=f"q{t}", name=f"q{t}")
                    for t in range(CP)]
            k_f8 = [qkvp.tile([P, 2, N], F8, tag=f"k{t}", name=f"k{t}")
                    for t in range(CP)]
            v_f8 = [qkvp.tile([P, 2, C], F8, tag=f"v{j}", name=f"v{j}")
                    for j in range(JP)]
            hT_f8 = [hTp.tile([P, 2, NQ], F8, tag=f"hT{t}", name=f"hT{t}")
                     for t in range(CP)]
            w_sb = {}
            for wname in ("wq", "wk", "wv", "wo"):
                w_sb[wname] = [wp.tile([P, 2, C], F8, tag=f"{wname}{t}",
                                       name=f"{wname}{t}") for t in range(CP)]

            # =========== Phase A: groupnorm -> hn (fp8) ===========
            with tc.tile_pool(name="gnsb", bufs=2) as gnp, \
                 tc.tile_pool(name="gnps", bufs=2, space="PSUM") as gnps:

                for ct in range(CT):
                    stats = gnp.tile([P, 8, 6], F32, tag="stats")
                    for s in range(8):
                        nc.sync.dma_start(
                            out=x_sb[ct][:, s * 512:(s + 1) * 512],
                            in_=x_t[ct][:, s * 512:(s + 1) * 512],
                        )
                        nc.vector.bn_stats(
                            out=stats[:, s, :], in_=x_sb[ct][:, s * 512:(s + 1) * 512]
                        )
                    if ct == 0:
                        # consts ride the DMA queue behind ct0's x chunks
                        nc.sync.dma_start(out=cvec_sb[:], in_=cvec_h)
                        nc.sync.dma_start(out=g8T_sb[:], in_=g8T_h)
                    mv = gnp.tile([P, 2], F32, tag="mv")
                    nc.vector.bn_aggr(out=mv[:], in_=stats[:])
                    cstat = gnp.tile([P, 2], F32, tag="cstat")
                    nc.vector.tensor_copy(cstat[:, 0:1], mv[:, 0:1])
                    nc.vector.tensor_mul(cstat[:, 1:2], mv[:, 0:1], mv[:, 0:1])
                    nc.vector.tensor_add(cstat[:, 1:2], cstat[:, 1:2], mv[:, 1:2])
                    psA = gnps.tile([NG, 2], F32, tag="gn")
                    nc.tensor.matmul(psA[:], lhsT=cvec_sb[:, 20:28], rhs=cstat[:],
                                     start=True, stop=True)
                    gt = gnp.tile([NG, 2], F32, tag="gt")
                    nc.vector.tensor_copy(gt[:], psA[:])
                    psB = gnps.tile([P, 2], F32, tag="gn")
                    nc.tensor.matmul(psB[:], lhsT=g8T_sb[:], rhs=gt[:],
                                     start=True, stop=True)
                    gstat = gnp.tile([P, 2], F32, tag="gstat")
                    nc.vector.tensor_copy(gstat[:], psB[:])
                    vtmp = gnp.tile([P, 1], F32, tag="vtmp")
                    nc.vector.tensor_mul(vtmp[:], gstat[:, 0:1], gstat[:, 0:1])
                    nc.vector.tensor_tensor(
                        out=vtmp[:], in0=gstat[:, 1:2], in1=vtmp[:],
                        op=mybir.AluOpType.subtract,
                    )
                    nc.scalar.activation(
                        out=vtmp[:], in_=vtmp[:],
                        func=mybir.ActivationFunctionType.Sqrt,
                        bias=eps_t[:], scale=1.0,
                    )
                    rstd = gnp.tile([P, 1], F32, tag="rstd")
                    nc.vector.reciprocal(out=rstd[:], in_=vtmp[:])
                    a_t = gnp.tile([P, 1], F32, tag="a_t")
                    nc.vector.tensor_mul(a_t[:], rstd[:], cvec_sb[:, 12 + ct:13 + ct])
                    d_t = gnp.tile([P, 1], F32, tag="d_t")
                    nc.vector.tensor_mul(d_t[:], gstat[:, 0:1], a_t[:])
                    nc.vector.tensor_tensor(
                        out=d_t[:], in0=cvec_sb[:, 16 + ct:17 + ct], in1=d_t[:],
                        op=mybir.AluOpType.subtract,
                    )
                    # apply split ACT/DVE so the last tile's apply is short
                    hdst = hn_f8[ct // 2]
                    nc.scalar.activation(
                        out=hdst[:, ct % 2, 0:2048],
                        in_=x_sb[ct][:, 0:2048],
                        func=IDENT, scale=a_t[:], bias=d_t[:],
                    )
                    nc.vector.tensor_scalar(
                        out=hdst[:, ct % 2, 2048:4096],
                        in0=x_sb[ct][:, 2048:4096],
                        scalar1=a_t[:], scalar2=d_t[:],
                        op0=MULT, op1=ADD,
                    )

            # deferred weight loads (after x so groupnorm owns DMA at t=0)
            wq_t = wq_h.rearrange("(t p) o -> t p o", p=P)
            wk_t = wk_h.rearrange("(t p) o -> t p o", p=P)
            wv_t = wv_h.rearrange("(t p) o -> t p o", p=P)
            wo_t = wo_h.rearrange("(t p) o -> t p o", p=P)
            for t in range(CP):
                for s in range(2):
                    nc.sync.dma_start(out=w_sb["wq"][t][:, s, :], in_=wq_t[2 * t + s])
                    nc.sync.dma_start(out=w_sb["wk"][t][:, s, :], in_=wk_t[2 * t + s])
                    nc.sync.dma_start(out=w_sb["wv"][t][:, s, :], in_=wv_t[2 * t + s])
                    nc.sync.dma_start(out=w_sb["wo"][t][:, s, :], in_=wo_t[2 * t + s])

            # =========== Phase B: k/q projections (fp8 DoubleRow) ===========
            # Per-engine psum pools (ACT and DVE drain their own rings in
            # parallel); 2-bank tiles pairing adjacent token slices of the
            # same out-channel block so the evac is one wide instruction.
            with tc.tile_pool(name="pjA", bufs=2, space="PSUM") as pjA, \
                 tc.tile_pool(name="pjD", bufs=2, space="PSUM") as pjD:

                def proj_pair(idx, wname, osl2, co, dst, bcol):
                    on_act = idx % 2 == 0
                    pool = pjA if on_act else pjD
                    ps = pool.tile([P, 1024], F32, tag="pj")
                    for h_ in range(2):
                        for t in range(CP):
                            nc.tensor.matmul(
                                ps[:, h_ * 512:(h_ + 1) * 512],
                                lhsT=w_sb[wname][t][:, :, co * P:(co + 1) * P],
                                rhs=hn_f8[t][:, :, (osl2 * 2 + h_) * 512:
                                             (osl2 * 2 + h_ + 1) * 512],
                                start=(t == 0), stop=(t == CP - 1),
                                perf_mode=DR,
                            )
                    if on_act:
                        nc.scalar.activation(out=dst, in_=ps[:], func=IDENT,
                                             bias=bcol, scale=1.0)
                    else:
                        nc.vector.tensor_scalar_add(out=dst, in0=ps[:],
                                                    scalar1=bcol)

                ei = 0
                # k for all 4096 keys, (jsl2, co) order so early score tiles
                # unblock first; then q
                for jsl2 in range(N // 1024):
                    for co in range(CT):
                        proj_pair(ei, "wk", jsl2, co,
                                  k_f8[co // 2][:, co % 2, jsl2 * 1024:(jsl2 + 1) * 1024],
                                  cvec_sb[:, 4 + co:5 + co])
                        ei += 1
                for isl2 in range(NQ // 1024):
                    for co in range(CT):
                        proj_pair(ei, "wq", isl2, co,
                                  q_f8[co // 2][:, co % 2, isl2 * 1024:(isl2 + 1) * 1024],
                                  cvec_sb[:, 0 + co:1 + co])
                        ei += 1

            # =========== Phase C: attention pipeline ===========
            # pT is double-buffered across strips so the ACT exp stream never
            # waits for consumers. Strip st's l and h matmuls run inside
            # strip st+1's score window; h accumulates CB-MAJOR (one output
            # channel block at a time over all 16 resident pT pairs), which
            # needs only a 2-bank ping-pong instead of 4 held banks. The v
            # projection hides inside strip 0's window; its psum pool closes
            # before the h pools open so everything fits in 8 banks.
            with tc.tile_pool(name="scA", bufs=2, space="PSUM") as scA, \
                 tc.tile_pool(name="attn", bufs=1) as ap_, \
                 tc.tile_pool(name="lsb", bufs=2) as lsp, \
                 tc.tile_pool(name="outt", bufs=3) as otp:

                # two pT sets (strip parity)
                pT = [[ap_.tile([P, 2, 512], F8, tag=f"pT{s}_{j}",
                                name=f"pT{s}_{j}") for j in range(JP)]
                      for s in range(2)]

                def sc_slot(st, jp):
                    """One 2-bank score pair tile + its exp."""
                    i0 = st * 512
                    sc = scA.tile([P, 1024], F32, tag="scA",
                                  name=f"s{st}_{jp}")
                    for h_ in range(2):
                        for t in range(CP):
                            nc.tensor.matmul(
                                sc[:, h_ * 512:(h_ + 1) * 512],
                                lhsT=k_f8[t][:, :, (2 * jp + h_) * P:(2 * jp + h_ + 1) * P],
                                rhs=q_f8[t][:, :, i0:i0 + 512],
                                start=(t == 0), stop=(t == CP - 1),
                                perf_mode=DR,
                            )
                    nc.scalar.activation(
                        out=pT[st % 2][jp][:], in_=sc[:],
                        func=EXP, scale=SCALE, bias=ebias_t[:],
                    )

                def aux_v(pjV):
                    """v projection: matmuls on PE, scaled-copy evac on DVE
                    (bv folded into bo on the host)."""
                    for jp in range(JP):
                        ps = pjV.tile([P, 1024], F32, tag="pv", name=f"v{jp}")
                        for m in range(2):
                            for t in range(CP):
                                yield nc.tensor.matmul(
                                    ps[:, m * 512:(m + 1) * 512],
                                    lhsT=hn_f8[t][:, :, (2 * jp + m) * P:(2 * jp + m + 1) * P],
                                    rhs=w_sb["wv"][t][:],
                                    start=(t == 0), stop=(t == CP - 1),
                                    perf_mode=DR,
                                )
                        nc.vector.tensor_scalar_mul(out=v_f8[jp][:], in0=ps[:],
                                                    scalar1=V_SCALE)

                def aux_lh(st, hp, lpool):
                    """Deferred work for strip st (runs in strip st+1's
                    window): l-run, rl, rlb, then cb-major h runs with
                    normalized fp8 evacs."""
                    i0 = st * 512
                    pts = pT[st % 2]
                    lt = lpool.tile([1, 512], F32, tag="l", name=f"l{st}")
                    for jp in range(JP):
                        yield nc.tensor.matmul(
                            lt[:], lhsT=ones_f8[:, :, 0:1], rhs=pts[jp][:],
                            start=(jp == 0), stop=(jp == JP - 1),
                            perf_mode=DR,
                        )
                    rl1 = lsp.tile([1, 512], F32, tag="rl1", name=f"rl1{st}")
                    nc.vector.reciprocal(out=rl1[:], in_=lt[:])
                    rlb = lsp.tile([P, 512], F32, tag="rlb", name=f"rlb{st}")
                    nc.gpsimd.partition_broadcast(rlb[:], rl1[:])
                    for cb in range(CT):
                        hps = hp.tile([P, 512], F32, tag="h",
                                      name=f"hps{st}_{cb}")
                        for jp in range(JP):
                            yield nc.tensor.matmul(
                                hps[:],
                                lhsT=v_f8[jp][:, :, cb * P:(cb + 1) * P],
                                rhs=pts[jp][:],
                                start=(jp == 0), stop=(jp == JP - 1),
                                perf_mode=DR,
                            )
                        nc.vector.tensor_mul(
                            hT_f8[cb // 2][:, cb % 2, i0:i0 + 512],
                            hps[:], rlb[:],
                        )

                def strip_out(st, hp):
                    """out-projection + bias + residual + store (generator
                    so it can weave between score slots instead of blocking
                    the strip boundary)."""
                    i0 = st * 512
                    for co in range(CT):
                        ps = hp.tile([P, 512], F32, tag="h", name=f"op{st}_{co}")
                        for t in range(CP):
                            yield nc.tensor.matmul(
                                ps[:],
                                lhsT=w_sb["wo"][t][:, :, co * P:(co + 1) * P],
                                rhs=hT_f8[t][:, :, i0:i0 + 512],
                                start=(t == 0), stop=(t == CP - 1),
                                perf_mode=DR,
                            )
                        ot = otp.tile([P, 512], F32, tag="ot")
                        nc.vector.scalar_tensor_tensor(
                            out=ot[:], in0=ps[:], scalar=cvec_sb[:, 8 + co:9 + co],
                            in1=x_sb[co][:, i0:i0 + 512], op0=ADD, op1=ADD,
                        )
                        nc.sync.dma_start(
                            out=out_t[co][:, i0:i0 + 512], in_=ot[:]
                        )

                def chain(*gens):
                    for g in gens:
                        yield from g

                def weave(st, aux_gen):
                    """Emit strip st's 16 score slots with ~5 aux PE ops
                    between consecutive slots."""
                    for jp in range(JP):
                        sc_slot(st, jp)
                        if aux_gen is not None:
                            for _ in range(6 if st == 0 else 5):
                                if next(aux_gen, None) is None:
                                    aux_gen = None
                                    break
                    while aux_gen is not None and next(aux_gen, None) is not None:
                        pass

                # strip 0 (v hides in its window; pjV closes right after)
                pjV_cm = tc.tile_pool(name="pjV", bufs=2, space="PSUM")
                pjV = pjV_cm.__enter__()
                weave(0, aux_v(pjV))
                pjV_cm.__exit__(None, None, None)

                hp_cm = tc.tile_pool(name="hacc", bufs=2, space="PSUM")
                hp = hp_cm.__enter__()
                lp_cm = tc.tile_pool(name="lps", bufs=1, space="PSUM")
                lpool = lp_cm.__enter__()

                for st in range(1, NSTRIP):
                    gens = [aux_lh(st - 1, hp, lpool)]
                    if st >= 2:
                        gens = [strip_out(st - 2, hp)] + gens
                    weave(st, chain(*gens))
                # drain: out-proj of strip 2, then last strip's deferred work
                for _ in chain(strip_out(NSTRIP - 2, hp),
                               aux_lh(NSTRIP - 1, hp, lpool)):
                    pass
                for _ in strip_out(NSTRIP - 1, hp):
                    pass

                lp_cm.__exit__(None, None, None)
                hp_cm.__exit__(None, None, None)

            if debug:
                for t in range(CP):
                    nc.sync.dma_start(out=dbg["hn"][t], in_=hn_f8[t][:])
                    nc.sync.dma_start(out=dbg["q"][t], in_=q_f8[t][:])
                    nc.sync.dma_start(out=dbg["k"][t], in_=k_f8[t][:])
                    nc.sync.dma_start(out=dbg["hT"][t], in_=hT_f8[t][:])
                for jp in range(JP):
                    nc.sync.dma_start(out=dbg["v"][jp], in_=v_f8[jp][:])

    nc.finalize()
    return nc


def kernel(**inputs):
    if "nc" not in _CACHE:
        _CACHE["nc"] = build_bass()
    nc = _CACHE["nc"]

    x = np.ascontiguousarray(np.asarray(inputs["x"], dtype=np.float32))
    B = x.shape[0]
    xf = x.reshape(B, C, N)

    def f8T(w, scale=1.0):
        return np.ascontiguousarray(
            (np.asarray(w, dtype=np.float32).T * scale).astype(
                ml_dtypes.float8_e4m3)
        )

    # softmax weights sum to 1, so the v bias rides through attention:
    # h = p@(v0+bv)/l = p@v0/l + bv  =>  fold wo@bv into bo (exact, fp32)
    wo32 = np.asarray(inputs["wo"], np.float32)
    bo_eff = (np.asarray(inputs["bo"], np.float32)
              + wo32 @ np.asarray(inputs["bv"], np.float32))

    def colsT(v):
        return np.asarray(v, np.float32).reshape(CT, P).T

    g8_np = np.zeros((P, 8), np.float32)
    for c in range(P):
        g8_np[c, c // 16] = 1.0 / 16
    cvec = np.concatenate([
        colsT(inputs["bq"]), colsT(inputs["bk"]), colsT(bo_eff),
        colsT(inputs["norm_g"]), colsT(inputs["norm_b"]), g8_np,
    ], axis=1)

    shared = {
        "wqT": f8T(inputs["wq"]), "wkT": f8T(inputs["wk"]),
        "wvT": f8T(inputs["wv"]), "woT": f8T(inputs["wo"], 1.0 / V_SCALE),
        "cvec": np.ascontiguousarray(cvec, dtype=np.float32),
    }

    in_maps = []
    for core in range(2 * B):
        b, half = core // 2, core % 2
        xb = xf[b]
        if half:
            xb = np.concatenate([xb[:, NQ:], xb[:, :NQ]], axis=1)
        in_maps.append({"x": np.ascontiguousarray(xb), **shared})

    import os
    trace = bool(os.environ.get("BASS_KERNEL_TRACE"))
    res = run_bass_kernel_spmd(
        nc, in_maps, core_ids=list(range(2 * B)), trace=trace,
        trace_cores=list(range(2 * B)) if trace else None,
    )
    _CACHE["last_results"] = res

    out = np.empty((B, C, N), np.float32)
    for core in range(2 * B):
        b, half = core // 2, core % 2
        out[b][:, half * NQ:(half + 1) * NQ] = res.results[core]["out"]
    return out.reshape(B, C, 64, 64)
